# revision 7
# baseline (speedup 1.0000x reference)
"""Trainium2 Bass kernel for nn_IngredientScannerLoss.

Per row (12 coords = 6 (x,y) pairs):
    delta = output - target
    dist_j = sqrt(dx_j^2 + dy_j^2)
    n_j    = (s0_j*dx_j > 0) + (s1_j*dy_j > 0)   (sign-gated count, 0/1/2)
    f(x)   = ((x+1)^1.2 - 1)*2
    t_j    = [dist, f(dist), f(f(dist))][n_j]
    loss   = sum_j t_j

Data-parallel over 8 NeuronCores: rows split 8 x 500_000, each shard
zero-padded to 501_760 = 128*3920 rows so tiles are [128, RT*12].

Engine split per tile:
    GPSIMD: delta = a - b                       (tensor_tensor subtract)
    DVE:    s = dx^2+dy^2 (custom op), n (custom op x6 pair columns),
            d1/d2 affines, predicated selects, row-sum reduce
    ACT:    ln/exp chains (single natural_log_exp table set; sqrt is done
            as exp(0.5*ln s) to avoid table switches)
"""

import numpy as np

import concourse.bacc as bacc
import concourse.bass as bass
import concourse.mybir as mybir
import concourse.tile as tile
from concourse import dve_ops
from concourse.bass_utils import run_bass_kernel_spmd
from concourse.dve_ops import DveOp
from concourse.dve_spec import Spec, Src0, Src1, C0, C1, Zero, _has_src1, lower, sq
from concourse.dve_uop import DveOpSpec

P = 128
COLS = 12
NPAIR = 6
B = 4_000_000
N_CORES = 8
ROWS_VALID = B // N_CORES          # 500_000
RT = 245                           # rows per partition per tile
NT = 16                            # tiles per core
ROWS_PC = P * RT * NT              # 501_760 padded rows per core

# per-coordinate condition signs (see reference _SIGNS)
SIGNS = [1.0, 1.0, 1.0, -1.0, -1.0, -1.0, -1.0, 1.0, 0.0, 1.0, 0.0, -1.0]

F32 = mybir.dt.float32
AF = mybir.ActivationFunctionType
ALU = mybir.AluOpType

# how many pair columns can ever hit n == 2 (pairs 4,5 have s0 == 0 -> n <= 1,
# so the second transform is only needed for pair columns 0..3)
NPAIR2 = 4

# ---------------------------------------------------------------- custom ops


def _register_op(name: str, spec: Spec, subdim: bool = False) -> DveOp:
    for op in dve_ops.OPS:
        if op.name == name:
            return op
    if name not in dve_ops._SUB_OPCODE_FOR_NAME:
        row = max(dve_ops._SUB_OPCODE_FOR_NAME.values()) + 1
        assert row < 0x20, "custom DVE opcode rows exhausted"
        dve_ops._SUB_OPCODE_FOR_NAME[name] = row
    shas = {}
    for ver in ("v3", "v4"):
        try:
            shas[ver] = DveOpSpec(
                name=name,
                opcode=dve_ops.get_dve_sub_opcode(name),
                uops=lower(spec, ver=ver),
                rd1_en=_has_src1(spec),
            ).sha(ver)
        except Exception:
            pass
    op = DveOp(name, spec, subdim, shas)
    dve_ops.OPS.append(op)
    dve_ops.CUSTOM_DVE_SPECS[name] = spec
    return op


# s = in0^2 + in1^2  (in0/in1 = even/odd delta columns)
PAIRDIST = _register_op(
    "ANT_PAIRDIST",
    Spec(
        body=sq(Src0) + sq(Src1),
        reference=lambda in0, in1, s0, s1, imm2: (
            in0.astype(np.float32) ** 2 + in1.astype(np.float32) ** 2
        ),
    ),
)

# n = (in0*s0 > 0) + (in1*s1 > 0)
CGATE = _register_op(
    "ANT_CGATE",
    Spec(
        body=(Src0 * C0 > Zero) + (Src1 * C1 > Zero),
        reference=lambda in0, in1, s0, s1, imm2: (
            ((in0.astype(np.float32) * s0) > 0).astype(np.float32)
            + ((in1.astype(np.float32) * s1) > 0).astype(np.float32)
        ),
    ),
)


# ---------------------------------------------------------------- bass build


def build_nc(rt: int = RT, nt: int = NT):
    """Build the single-core SPMD program for [P*rt*nt, 12] inputs."""
    rows = P * rt * nt
    nc = bacc.Bacc("TRN2", debug=False, target_bir_lowering=False,
                   num_devices=N_CORES)
    # activation bias=-1.0 needs a registered const AP (only 0.0/1.0 ship)
    if (F32, -1.0) not in nc.const_aps.aps:
        cm1 = nc.alloc_sbuf_tensor("const-float32-m1", [P, 1], F32)
        nc.gpsimd.memset(cm1.ap(), -1.0)
        nc.const_aps.aps[(F32, -1.0)] = cm1.ap()
        nc.all_engine_barrier()
    a = nc.dram_tensor("output", [rows, COLS], F32, kind="ExternalInput").ap()
    b = nc.dram_tensor("target", [rows, COLS], F32, kind="ExternalInput").ap()
    o = nc.dram_tensor("loss", [rows], F32, kind="ExternalOutput").ap()

    a3 = a.rearrange("(n p r) m -> n p (r m)", p=P, r=rt)
    b3 = b.rearrange("(n p r) m -> n p (r m)", p=P, r=rt)
    o3 = o.rearrange("(n p r) -> n p r", p=P, r=rt)

    with tile.TileContext(nc) as tc:
        with tc.tile_pool(name="sb", bufs=2) as pool:
            for i in range(nt):
                ta = pool.tile([P, rt * COLS], F32, tag="ta")
                nc.sync.dma_start(out=ta[:], in_=a3[i])
                tb = pool.tile([P, rt * COLS], F32, tag="tb")
                nc.sync.dma_start(out=tb[:], in_=b3[i])

                delta = pool.tile([P, rt * COLS], F32, tag="delta")
                nc.gpsimd.tensor_tensor(delta[:], ta[:], tb[:], ALU.subtract)

                # pair-major views of delta: element (j, r) at 12r + 2j (+1)
                dv = delta[:].rearrange("p (r j two) -> p j r two",
                                        j=NPAIR, two=2)

                # All per-pair intermediates use pair-major layout [P, j, r]
                # so "pairs 0..3" slices are contiguous prefixes [P, 0:4*rt].

                # s = dx^2 + dy^2, [P, 6*rt] pair-major
                s = pool.tile([P, rt * NPAIR], F32, tag="s")
                nc.vector._custom_dve(PAIRDIST, out=s[:], in0=dv[:, :, :, 0],
                                      in1=dv[:, :, :, 1])

                # n gates, [P, 6*rt] pair-major
                n = pool.tile([P, rt * NPAIR], F32, tag="n")
                n3 = n[:].rearrange("p (j r) -> p j r", j=NPAIR)
                for j in range(NPAIR):
                    nc.vector._custom_dve(
                        CGATE,
                        out=n3[:, j, :],
                        in0=dv[:, j, :, 0],
                        in1=dv[:, j, :, 1],
                        s0=SIGNS[2 * j],
                        s1=SIGNS[2 * j + 1],
                    )

                # ACT chain (all funcs in natural_log_exp_and_others):
                # dist = exp(0.5*ln(s)); u = (dist+1)^1.2; v = (2u-1)^1.2
                lt = pool.tile([P, rt * NPAIR], F32, tag="lt")
                nc.scalar.activation(lt[:], s[:], AF.Ln)
                res = pool.tile([P, rt * NPAIR], F32, tag="res")
                nc.scalar.activation(res[:], lt[:], AF.Exp, scale=0.5)  # dist
                t = pool.tile([P, rt * NPAIR], F32, tag="t")
                nc.scalar.activation(t[:], res[:], AF.Ln, bias=1.0)
                u = pool.tile([P, rt * NPAIR], F32, tag="u")
                nc.scalar.activation(u[:], t[:], AF.Exp, scale=1.2)

                # second transform only needed where n can be 2: pairs 0..3
                w4 = rt * NPAIR2
                t2 = pool.tile([P, w4], F32, tag="t2")
                nc.scalar.activation(t2[:], u[:, 0:w4], AF.Ln,
                                     scale=2.0, bias=-1.0)
                v = pool.tile([P, w4], F32, tag="v")
                nc.scalar.activation(v[:], t2[:], AF.Exp, scale=1.2)

                # d1 = 2u - 2 (all 6 pairs), d2 = 2v - 2 (pairs 0..3)
                d1 = pool.tile([P, rt * NPAIR], F32, tag="d1")
                nc.vector.tensor_scalar(d1[:], u[:], 2.0, -2.0, ALU.mult,
                                        ALU.add)
                d2 = pool.tile([P, w4], F32, tag="d2")
                nc.vector.tensor_scalar(d2[:], v[:], 2.0, -2.0, ALU.mult,
                                        ALU.add)

                # res (= dist) overwritten by d1 where n>=1, d2 where n>=2.
                # CopyPredicated wants an integer mask; fp32 {0.,1.,2.}
                # bitcast to int32 is nonzero exactly where the float is.
                I32 = mybir.dt.int32
                nc.vector.copy_predicated(res[:], n[:].bitcast(I32), d1[:])
                m2 = pool.tile([P, w4], F32, tag="m2")
                nc.vector.tensor_scalar(m2[:], n[:, 0:w4], 1.0, 0.0,
                                        ALU.subtract, ALU.max)
                nc.vector.copy_predicated(res[:, 0:w4], m2[:].bitcast(I32),
                                          d2[:])

                # row sums: iterate rows outer, pairs inner
                res_rj = res[:].rearrange("p (j r) -> p r j", j=NPAIR)
                ot = pool.tile([P, rt], F32, tag="ot")
                nc.vector.tensor_reduce(ot[:], res_rj, axis=mybir.AxisListType.X,
                                        op=ALU.add)
                nc.sync.dma_start(out=o3[i], in_=ot[:])
    nc.compile()
    return nc


_NC_CACHE: dict = {}


def _get_nc(rt: int = RT, nt: int = NT):
    key = (rt, nt)
    if key not in _NC_CACHE:
        _NC_CACHE[key] = build_nc(rt, nt)
    return _NC_CACHE[key]


# ---------------------------------------------------------------- entrypoint


def kernel(output, target):
    a = np.asarray(output, dtype=np.float32)
    b = np.asarray(target, dtype=np.float32)
    assert a.shape == (B, COLS) and b.shape == (B, COLS)

    a_sh = np.zeros((N_CORES, ROWS_PC, COLS), dtype=np.float32)
    b_sh = np.zeros((N_CORES, ROWS_PC, COLS), dtype=np.float32)
    a_sh[:, :ROWS_VALID, :] = a.reshape(N_CORES, ROWS_VALID, COLS)
    b_sh[:, :ROWS_VALID, :] = b.reshape(N_CORES, ROWS_VALID, COLS)

    nc = _get_nc()
    in_maps = [
        {"output": a_sh[c], "target": b_sh[c]} for c in range(N_CORES)
    ]
    r = run_bass_kernel_spmd(nc, in_maps, list(range(N_CORES)))
    out = np.empty((N_CORES, ROWS_VALID), dtype=np.float32)
    for c in range(N_CORES):
        out[c] = r.results[c]["loss"][:ROWS_VALID]
    return out.reshape(B)
